# revision 6
# baseline (speedup 1.0000x reference)
"""BlobbyGyroid TRN2 Bass kernel.

Computes, for P=1048576 points, the warped-gyroid + metaballs + harmonics
field F(p), its analytic gradient wrt p, and the derived outputs
(sigma, rgb, Fval, n) of the reference nn.Module — data-parallel over 8
NeuronCores (points axis sharded, parameters baked into the program as
immediates/const-table at trace time).

Self-contained: only needs numpy + the concourse (Bass) stack available in
the container.
"""
import math
import numpy as np
from contextlib import ExitStack

import concourse.bass as bass
import concourse.bacc as bacc
import concourse.tile as tile
from concourse import mybir
from concourse.bass_utils import run_bass_kernel_spmd

AF = mybir.ActivationFunctionType
ALU = mybir.AluOpType
F32 = mybir.dt.float32

P_TOTAL = 1048576
NCORES = 8
PPC = P_TOTAL // NCORES          # 131072 points per core
PARTS = 128
J, N, K = 4, 16, 8

TWO_PI = 2.0 * math.pi
INV_2PI = 1.0 / TWO_PI
MAGIC = float(1.5 * 2 ** 23)     # round-to-nearest via add/sub
MB_GRP = 8                       # metaballs per table-set group

_CACHE = {}


def _softplus(x):
    x = np.asarray(x, np.float64)
    return np.log1p(np.exp(-np.abs(x))) + np.maximum(x, 0.0)


class Consts:
    """Bias-constant table: host array -> broadcast [128, n] SBUF tile."""

    def __init__(self):
        self.vals = []
        self.idx = {}

    def col(self, val):
        val = float(np.float32(val))
        if val not in self.idx:
            self.idx[val] = len(self.vals)
            self.vals.append(val)
        return self.idx[val]

    def ap(self, tile_, val):
        i = self.idx[float(np.float32(val))]
        return tile_[:, i:i + 1]


def build(params, fd, n_tiles, repeat=1):
    """Build the per-core SPMD program. params: dict of np arrays (full
    precision host params). fd: free-dim columns per tile. n_tiles:
    tiles per core (fd * 128 * n_tiles == points per core)."""
    ppc = fd * PARTS * n_tiles

    # ---- host-side parameter prep (f64 for exactness, f32 immediates) ----
    wa = np.asarray(params["warp_a"], np.float64)           # [J,3]
    wB = np.asarray(params["warp_B"], np.float64)           # [J,3,3]
    wom = np.asarray(params["warp_omega"], np.float64)      # [J]
    wph = np.asarray(params["warp_phi"], np.float64)        # [J,3]
    gom = np.asarray(params["omega"], np.float64)           # [3]
    gph = np.asarray(params["phi"], np.float64)             # [3]
    alpha = float(params["alpha"])
    mw = np.asarray(params["mb_w"], np.float64)             # [N]
    beta = _softplus(np.asarray(params["mb_beta_raw"])) + 0.3
    cbar = np.asarray(params["mb_cbar"], np.float64)        # [N,3]
    mu = np.asarray(params["mb_u"], np.float64)             # [N,3]
    nu = np.asarray(params["mb_nu"], np.float64)            # [N]
    psi = np.asarray(params["mb_psi"], np.float64)          # [N]
    hs = np.asarray(params["h_s"], np.float64)              # [K]
    hk = np.asarray(params["h_k"], np.float64)              # [K,3]
    hw = np.asarray(params["h_w"], np.float64)              # [K]
    hz = np.asarray(params["h_zeta"], np.float64)           # [K]
    kappa = float(params["kappa"])
    bias_b = float(params["bias_b"])
    delta = float(_softplus(params["delta_raw"]) + 0.001)
    q0 = np.asarray(params["q0"], np.float64)
    q1 = float(params["q1"])
    Q2 = np.asarray(params["Q2"], np.float64)               # [3,3]
    q3 = float(params["q3"])
    eta = float(params["eta"])
    wv = np.asarray(params["w_vec"], np.float64)            # [3]
    zeta = float(params["zeta"])
    ld = np.asarray(params["light_dir"], np.float64)        # [3]

    assert np.all(mw > 0), "kernel assumes positive metaball weights"

    nc = bacc.Bacc(None, target_bir_lowering=False)

    pp = nc.declare_dram_parameter("pp", [ppc, 3], F32, isOutput=False)
    tt_d = nc.declare_dram_parameter("td", [ppc, 1], F32, isOutput=False)
    o_sig = nc.declare_dram_parameter("o_sig", [ppc, 1], F32, isOutput=True)
    o_rgb = nc.declare_dram_parameter("o_rgb", [ppc, 3], F32, isOutput=True)
    o_f = nc.declare_dram_parameter("o_f", [ppc], F32, isOutput=True)
    o_n = nc.declare_dram_parameter("o_n", [ppc, 3], F32, isOutput=True)

    # ---- constant-bias table ----
    cst = Consts()
    cst.col(math.pi / 2)
    for i in range(N):
        for e in range(3):
            cst.col(-cbar[i, e])
        cst.col(math.log(mw[i]))
    cst_d = nc.declare_dram_parameter("cst", [1, max(len(cst.vals), 1)], F32,
                                      isOutput=False)

    ppr = pp[:, :].rearrange("(s k i) e -> s k (i e)", s=n_tiles, k=PARTS)
    ttr = tt_d[:, :].rearrange("(s k i) e -> s k (i e)", s=n_tiles, k=PARTS)
    sigr = o_sig[:, :].rearrange("(s k i) e -> s k (i e)", s=n_tiles, k=PARTS)
    rgbr = o_rgb[:, :].rearrange("(s k i) e -> s k (i e)", s=n_tiles, k=PARTS)
    fr = o_f[:].rearrange("(s k i) -> s k i", s=n_tiles, k=PARTS)
    nr = o_n[:, :].rearrange("(s k i) e -> s k (i e)", s=n_tiles, k=PARTS)

    with tile.TileContext(nc) as tc, ExitStack() as ctx:
        pool = ctx.enter_context(tc.tile_pool(name="pl", bufs=1))
        tph = ctx.enter_context(tc.tile_pool(name="tph", bufs=2))
        tpc = ctx.enter_context(tc.tile_pool(name="tpc", bufs=1))
        ppool = ctx.enter_context(tc.tile_pool(name="ps", bufs=1,
                                               space="PSUM"))
        HOT = {"lin", "rr", "frac", "af", "s", "ei", "Ei", "Fi",
               "rp0", "rp1", "rp2"}

        ctile = pool.tile([PARTS, len(cst.vals)], F32, tag="cst", name="cst")
        cst_b = bass.AP(tensor=cst_d[:, :].tensor, offset=cst_d[:, :].offset,
                        ap=[[0, PARTS], cst_d[:, :].ap[1]])
        nc.sync.dma_start(out=ctile, in_=cst_b)

        def P(tag):
            return pool.tile([PARTS, fd], F32, tag=tag, name=tag)

        def T(tag):
            pl = tph if tag in HOT else tpc
            return pl.tile([PARTS, fd], F32, tag=tag, name=tag)

        def ts(out, in0, s1, s2=None, op0=ALU.mult, op1=ALU.add):
            if s2 is None:
                nc.vector.tensor_scalar(out, in0, float(s1), None, op0)
            else:
                nc.vector.tensor_scalar(out, in0, float(s1), float(s2),
                                        op0, op1)

        def stt(out, in0, s, in1, op0=ALU.mult, op1=ALU.add):
            nc.vector.scalar_tensor_tensor(out, in0, float(s), in1, op0, op1)

        def tt(out, a, b, op=ALU.add, eng=None):
            (eng or nc.vector).tensor_tensor(out=out, in0=a, in1=b, op=op)

        def act(out, in_, func, scale=1.0, bias=0.0):
            nc.scalar.activation(out, in_, func, bias=bias, scale=float(scale))

        def cb(v):
            return cst.ap(ctile, v)

        def rround(out, lin):
            ts(out, lin, MAGIC, MAGIC, ALU.add, ALU.subtract)

        def frac_of(lin, rtag="rr", ftag="frac"):
            r = T(rtag)
            rround(r, lin)
            f = T(ftag)
            nc.vector.scalar_tensor_tensor(f, r, -1.0, lin, ALU.mult, ALU.add)
            return f

        def sin_of(f, out):
            act(out, f, AF.Sin, scale=TWO_PI)

        def cos_of(f, out, atag="af"):
            a = T(atag)
            act(a, f, AF.Abs)
            act(out, a, AF.Sin, scale=-TWO_PI, bias=cb(math.pi / 2))

        for s in list(range(n_tiles)) * repeat:
            big = pool.tile([PARTS, 3 * fd], F32, tag="big3", name="big3")
            ttile = P("tt")
            nc.sync.dma_start(out=big, in_=ppr[s])
            nc.sync.dma_start(out=ttile, in_=ttr[s])

            pxyz = []
            big_v = big.rearrange("k (i e) -> k i e", e=3)
            for e in range(3):
                pe = P(f"p{e}")
                nc.vector.tensor_copy(pe, big_v[:, :, e])
                pxyz.append(pe)

            # ---------------- warp (trig set) ----------------
            qq = [P(f"q{e}") for e in range(3)]
            cplanes = {}
            for j in range(J):
                for e in range(3):
                    lin = T("lin")
                    ts(lin, ttile, wom[j] * INV_2PI, wph[j, e] * INV_2PI)
                    for d in range(3):
                        stt(lin, pxyz[d], wB[j, e, d] * INV_2PI, lin)
                    f = frac_of(lin)
                    sv = T("s")
                    sin_of(f, sv)
                    base = pxyz[e] if j == 0 else qq[e]
                    stt(qq[e], sv, wa[j, e], base)
                    cje = P(f"c{j}{e}")
                    cos_of(f, cje)
                    cplanes[(j, e)] = cje

            # ---------------- gyroid (trig) ----------------
            gs, gc = [], []
            for e in range(3):
                lin = T("lin")
                ts(lin, qq[e], gom[e] * INV_2PI, gph[e] * INV_2PI)
                f = frac_of(lin)
                gse, gce = T(f"gs{e}"), T(f"gc{e}")
                sin_of(f, gse)
                cos_of(f, gce)
                gs.append(gse)
                gc.append(gce)
            Gp = P("G")
            tmp = T("tmp")
            tt(Gp, gs[0], gc[1], ALU.mult, nc.gpsimd)
            tt(tmp, gs[1], gc[2], ALU.mult, nc.gpsimd)
            tt(Gp, Gp, tmp, ALU.add, nc.gpsimd)
            tt(tmp, gs[2], gc[0], ALU.mult, nc.gpsimd)
            tt(Gp, Gp, tmp, ALU.add, nc.gpsimd)
            dga = []
            pairs = [((0, 1), (0, 2)), ((1, 2), (0, 1)), ((2, 0), (1, 2))]
            for e, ((ca, cbi), (sa, sb)) in enumerate(pairs):
                de = P(f"dga{e}")
                t2 = T("tmp")
                tt(de, gc[ca], gc[cbi], ALU.mult)
                tt(t2, gs[sa], gs[sb], ALU.mult)
                tt(de, de, t2, ALU.subtract)
                dga.append(de)

            # ---------------- sin(2q), emissive (trig) ----------------
            s2q = []
            for e in range(3):
                lin = T("lin")
                ts(lin, qq[e], 2.0 * INV_2PI, 0.0)
                f = frac_of(lin)
                se = P(f"s2q{e}")
                sin_of(f, se)
                s2q.append(se)
            lin = T("lin")
            ts(lin, ttile, zeta * INV_2PI, 0.0)
            for d in range(3):
                stt(lin, qq[d], eta * wv[d] * INV_2PI, lin)
            f = frac_of(lin)
            em = P("em")
            sin_of(f, em)

            # ---------------- harmonics (trig) ----------------
            Hp = P("H")
            gH = [P(f"gH{e}") for e in range(3)]
            for k in range(K):
                lin = T("lin")
                ts(lin, ttile, hw[k] * INV_2PI, hz[k] * INV_2PI)
                for d in range(3):
                    stt(lin, qq[d], hk[k, d] * INV_2PI, lin)
                f = frac_of(lin)
                sh, ch = T("sh"), T("ch")
                sin_of(f, sh)
                cos_of(f, ch)
                if k == 0:
                    ts(Hp, sh, hs[k], 0.0)
                    for e in range(3):
                        ts(gH[e], ch, hs[k] * hk[k, e], 0.0)
                else:
                    stt(Hp, sh, hs[k], Hp)
                    for e in range(3):
                        stt(gH[e], ch, hs[k] * hk[k, e], gH[e])

            # ---------------- metaballs, grouped ----------------
            Mp, P0, Sc = P("M"), P("P0"), [P(f"Sc{e}") for e in range(3)]
            for g0 in range(0, N, MB_GRP):
                idxs = range(g0, min(g0 + MB_GRP, N))
                sns, d2s = {}, {}
                # trig part
                for i in idxs:
                    lin = T("lin")
                    ts(lin, ttile, nu[i] * INV_2PI, psi[i] * INV_2PI)
                    f = frac_of(lin)
                    sn = ppool.tile([PARTS, fd], F32, tag=f"sn{i % MB_GRP}", name=f"sn{i % MB_GRP}")
                    sin_of(f, sn)
                    d2 = P(f"d2{i % MB_GRP}")
                    for e in range(3):
                        rp = T(f"rp{e}")
                        stt(rp, sn, -mu[i, e], qq[e])
                        sq = T(f"sq{e}")
                        act(sq, rp, AF.Square, bias=cb(-cbar[i, e]))
                        if e == 0:
                            nc.gpsimd.tensor_copy(d2, sq)
                        else:
                            tt(d2, d2, sq, ALU.add, nc.gpsimd)
                    sns[i], d2s[i] = sn, d2
                # exp part
                for i in idxs:
                    ei = T("ei")
                    act(ei, d2s[i], AF.Exp, scale=-beta[i],
                        bias=cb(math.log(mw[i])))
                    Ei = T("Ei")
                    ts(Ei, ei, beta[i], None)
                    Fi = T("Fi")
                    tt(Fi, Ei, sns[i], ALU.mult)
                    if i == 0:
                        nc.gpsimd.tensor_copy(Mp, ei)
                        nc.gpsimd.tensor_copy(P0, Ei)
                        for e in range(3):
                            ts(Sc[e], Ei, cbar[i, e], None)
                            stt(Sc[e], Fi, mu[i, e], Sc[e])
                    else:
                        tt(Mp, Mp, ei, ALU.add, nc.gpsimd)
                        tt(P0, P0, Ei, ALU.add, nc.gpsimd)
                        for e in range(3):
                            stt(Sc[e], Ei, cbar[i, e], Sc[e])
                            stt(Sc[e], Fi, mu[i, e], Sc[e])

            # ---------------- F, sigma (ln/exp set) ----------------
            Lp = P("L")
            act(Lp, Mp, AF.Ln, bias=1.0)
            inv = T("sh")
            act(inv, Lp, AF.Exp, scale=-1.0)
            Fp = P("F")
            stt(Fp, Gp, 0.1, Hp)
            stt(Fp, Lp, kappa, Fp)
            ts(Fp, Fp, 1.0, -bias_b - 0.1 * alpha)
            sig = P("sig")
            ax, en, l1, rx = T("ch"), T("rp0"), T("rp1"), T("rp2")
            act(ax, Fp, AF.Abs, scale=1.0 / delta)
            act(en, ax, AF.Exp, scale=-1.0)
            act(l1, en, AF.Ln, bias=1.0)
            act(rx, Fp, AF.Relu, scale=-1.0 / delta)
            tt(sig, l1, rx, ALU.add, nc.gpsimd)

            # ---------------- g_e and grad ----------------
            c2 = T("sq0")
            ts(c2, inv, -2.0 * kappa, None)
            gvec = []
            for e in range(3):
                ge = P(f"p{e}")
                tt(ge, qq[e], P0, ALU.mult)
                tt(ge, ge, Sc[e], ALU.subtract)
                tt(ge, ge, c2, ALU.mult)
                stt(ge, dga[e], 0.1 * gom[e], ge)
                tt(ge, ge, gH[e], ALU.add, nc.gpsimd)
                gvec.append(ge)
            # m_je = g_e * c_je (overwrite c planes in place)
            for j in range(J):
                for e in range(3):
                    cje = cplanes[(j, e)]
                    tt(cje, gvec[e], cje, ALU.mult)
            grad = []
            for d in range(3):
                gd = P(f"dga{d}")
                first = True
                for j in range(J):
                    for e in range(3):
                        kjed = wa[j, e] * wB[j, e, d]
                        base = gvec[d] if first else gd
                        stt(gd, cplanes[(j, e)], kjed, base)
                        first = False
                grad.append(gd)

            # ---------------- normalize, outputs ----------------
            S = T("sq1")
            nsq = T("sq2")
            act(S, grad[0], AF.Square)
            act(nsq, grad[1], AF.Square)
            tt(S, S, nsq, ALU.add, nc.gpsimd)
            act(nsq, grad[2], AF.Square)
            tt(S, S, nsq, ALU.add, nc.gpsimd)
            nc.vector.tensor_scalar_max(S, S, 1e-24)
            lnS = T("ei")
            act(lnS, S, AF.Ln)
            rn = T("Ei")
            act(rn, lnS, AF.Exp, scale=-0.5)
            nvec = []
            for d in range(3):
                tt(grad[d], grad[d], rn, ALU.mult)
                nvec.append(grad[d])

            nd = T("lin")
            ts(nd, nvec[0], ld[0], None)
            stt(nd, nvec[1], ld[1], nd)
            stt(nd, nvec[2], ld[2], nd)
            boost = T("rr")
            ts(boost, sig, 5.0, 1.0, ALU.mult, ALU.min)
            rgbp = []
            for e in range(3):
                A = T(f"gs{e}")
                ts(A, nd, q1, q0[e])
                for d in range(3):
                    stt(A, nvec[d], Q2[e, d], A)
                stt(A, em, q3, A)
                stt(A, s2q[e], 0.3, A)
                stt(A, boost, 0.5, A)
                rgbp.append(A)
            rgb_out = []
            for e in range(3):
                ro = T(f"gc{e}")
                act(ro, rgbp[e], AF.Sigmoid)
                rgb_out.append(ro)

            # ---------------- stores ----------------
            nc.sync.dma_start(out=sigr[s], in_=sig)
            nc.sync.dma_start(out=fr[s], in_=Fp)
            stage = pool.tile([PARTS, 3 * fd], F32, tag="big3b", name="big3b")
            st_v = stage.rearrange("k (i e) -> k i e", e=3)
            for e in range(3):
                nc.vector.tensor_copy(st_v[:, :, e], rgb_out[e])
            nc.sync.dma_start(out=rgbr[s], in_=stage)
            stage2 = pool.tile([PARTS, 3 * fd], F32, tag="big3", name="big3n")
            st2_v = stage2.rearrange("k (i e) -> k i e", e=3)
            for e in range(3):
                nc.vector.tensor_copy(st2_v[:, :, e], nvec[e])
            nc.sync.dma_start(out=nr[s], in_=stage2)

    nc.finalize()
    return nc, np.asarray(cst.vals, np.float32)[None, :]


def kernel(_repeat=1, **inputs):
    p = np.ascontiguousarray(np.asarray(inputs["p"], np.float32))
    t = np.ascontiguousarray(np.asarray(inputs["t"], np.float32))
    P = p.shape[0]
    ppc = P // NCORES
    fd = 512
    n_tiles = ppc // (PARTS * fd)
    assert n_tiles * PARTS * fd == ppc

    params = {k: np.asarray(v) for k, v in inputs.items()
              if k not in ("p", "t")}
    key = (P, _repeat,
           tuple(sorted((k, v.tobytes()) for k, v in params.items())))
    kh = hash(key)
    if kh not in _CACHE:
        _CACHE[kh] = build(params, fd, n_tiles, repeat=_repeat)
    nc, cst_arr = _CACHE[kh]

    in_maps = []
    for c in range(NCORES):
        sl = slice(c * ppc, (c + 1) * ppc)
        in_maps.append({"pp": p[sl], "td": t[sl], "cst": cst_arr})
    res = run_bass_kernel_spmd(nc, in_maps, core_ids=list(range(NCORES)))
    sig = np.concatenate([r["o_sig"] for r in res.results], axis=0)
    rgb = np.concatenate([r["o_rgb"] for r in res.results], axis=0)
    f = np.concatenate([r["o_f"] for r in res.results], axis=0)
    n = np.concatenate([r["o_n"] for r in res.results], axis=0)
    return sig, rgb, f, n
